# revision 25
# baseline (speedup 1.0000x reference)
"""Bass/Tile kernel for masked dot-product attention on 8 Trainium2 cores.

Problem: queries/keys/values [128, 1024, 64] fp32, valid_lens [128] int32.
  out[b] = softmax(mask(Q K^T / 8, valid_lens[b])) @ V

Design (v2):
  * Shard the 128 batch*heads across 8 cores, 16 head-slots per core.
    Heads sorted by valid_len desc, dealt round-robin -> one SPMD program.
  * Host pre-layout (numpy): Q^T/K^T transposed + fp16 cast on the host so
    the device needs no transposes at all.  Q^T is duplicated into both
    64-partition halves; K^T chunk pairs are packed into lower/upper halves
    so adjacent S matmuls row-tile (contraction=64 -> 2 concurrent).
  * V is packed per key-chunk as [128 keys, 64 V | 1.0] bf16 with the ones
    column baked in on the host; the PV matmul then yields [O | denom].
  * S^T = K_c Q^T per chunk on the PE -> PSUM [128 keys, 1024 q].
  * exp() is split between ScalarE (true Exp activation, masked via bias
    column) and VectorE (Schraudolph: one tensor_scalar computing the bf16
    BIT PATTERN of exp(s/8) as s*A + B[p] with int16 convert; masked keys
    get a bias that lands on a tiny positive bf16).  ~2x softmax throughput.
  * PV computes O directly in [q, d] layout: stationary = P^T q-tile
    [keys, 128], moving = [V_c | 1] -> accumulates [q, 64+1] in PSUM.
    No PE transposes, no epilogue copies; normalization is a reciprocal of
    the denom column + 8 per-partition-scalar multiplies.
  * Heads with valid_len == 0 are fixed up on the host (reference: uniform
    attention = mean of V).
"""

import math
from contextlib import ExitStack

import numpy as np
import ml_dtypes

import concourse.bass as bass  # noqa: F401
import concourse.mybir as mybir
import concourse.tile as tile
from concourse import bacc
from concourse.bass_utils import run_bass_kernel_spmd

BH, L, D = 128, 1024, 64
NCORES = 8
SLOTS = BH // NCORES  # 16
CHUNK = 128
NCH = L // CHUNK  # 8
F32 = mybir.dt.float32
F16 = mybir.dt.float16
BF16 = mybir.dt.bfloat16
I16 = mybir.dt.int16

# Schraudolph exp: bf16 bits of exp(s/8) ~= int16(s * SCH_A + SCH_B).
SCH_A = 0.125 * 1.4426950408889634 * 128.0  # 23.0831...
SCH_C = 6.0  # spread-centering correction (calibrated vs reference)
SCH_B = 127.0 * 128.0 - SCH_C
SCH_BMASK = 3100.0  # masked keys: packed in [~1540, ~4660] -> <2^-90, covers |s|<=134

DVE_FRAC = 0.40  # fraction of exp chunks routed to VectorE
PAIR_MODE = "none"  # 'all' | 'none' | 'alt' row-tiled S pairing

_program_cache: dict = {}


def _dve_pattern(m_list):
    """Deterministic ACT/DVE split per (slot, chunk), Bresenham on DVE_FRAC.

    The first two chunks of each head stay on ScalarE: their PV feeds the
    accumulator banks right when the previous head's epilogue occupies the
    VectorE queue, so a DVE exp there would serialize with it."""
    use = []
    err = 0.0
    for m in m_list:
        row = []
        for c in range(m):
            err += DVE_FRAC
            if err >= 1.0:
                err -= 1.0
                row.append(True)
            else:
                row.append(False)
        use.append(row)
    return use


def _build_program(m_list, full_list):
    dve = _dve_pattern(m_list)
    nc = bacc.Bacc("TRN2", target_bir_lowering=False, debug=False)
    qt_d = nc.dram_tensor("qt", [SLOTS, 128, L], F16, kind="ExternalInput").ap()
    kk_d = nc.dram_tensor("kk", [SLOTS, 128, 512], F16, kind="ExternalInput").ap()
    va_d = nc.dram_tensor("va", [SLOTS, 128, NCH * 65], BF16, kind="ExternalInput").ap()
    mb_d = nc.dram_tensor("mb", [128, SLOTS * NCH], F32, kind="ExternalInput").ap()
    wv_d = nc.dram_tensor("wv", [128, SLOTS * NCH], F32, kind="ExternalInput").ap()
    o_d = nc.dram_tensor("o", [SLOTS, L, D], F32, kind="ExternalOutput").ap()

    Exp = mybir.ActivationFunctionType.Exp
    Mult = mybir.AluOpType.mult
    Add = mybir.AluOpType.add

    with tile.TileContext(nc) as tc, ExitStack() as ctx:
        const = ctx.enter_context(tc.tile_pool(name="const", bufs=1))
        mb = const.tile([128, SLOTS * NCH], F32)
        wv = const.tile([128, SLOTS * NCH], F32)
        ones = const.tile([128, 1], F32)
        nc.gpsimd.memset(ones[:], 1.0)
        # Pre-load the exp table set so the first real activation is fast.
        actwarm = const.tile([128, 1], F32, tag="actwarm")
        nc.scalar.activation(actwarm[:], ones[:], Exp, bias=0.0, scale=1.0)
        warm = const.tile([128, 512], BF16, tag="warm")
        nc.vector.memset(warm[:], 0.5)

        qt_p = ctx.enter_context(tc.tile_pool(name="qt", bufs=4))
        kk_p = ctx.enter_context(tc.tile_pool(name="kk", bufs=4))
        va_p = ctx.enter_context(tc.tile_pool(name="va", bufs=4))
        pt_p = ctx.enter_context(tc.tile_pool(name="pt", bufs=8))
        osb_p = ctx.enter_context(tc.tile_pool(name="osb", bufs=2))
        rec_p = ctx.enter_context(tc.tile_pool(name="rec", bufs=2))

        # PSUM: 8 banks = s-ring 3 x [128,1024] (6) + two 1-bank accumulators.
        s_ps = ctx.enter_context(tc.tile_pool(name="s", bufs=3, space="PSUM"))
        a_ps = ctx.enter_context(tc.tile_pool(name="acc", bufs=1, space="PSUM"))

        def load_head(j):
            qt = qt_p.tile([128, L], F16, tag="qt", name=f"qt{j}")
            nc.sync.dma_start(qt[:], qt_d[j])
            kk = kk_p.tile([128, 512], F16, tag="kk", name=f"kk{j}")
            nc.sync.dma_start(kk[:], kk_d[j])
            va = va_p.tile([128, NCH * 65], BF16, tag="va", name=f"va{j}")
            nc.gpsimd.dma_start(va[:], va_d[j])
            return qt, kk, va

        PREFETCH = 3
        heads = {0: load_head(0)}
        nc.sync.dma_start(mb[:], mb_d[:])
        nc.sync.dma_start(wv[:], wv_d[:])
        for j in range(1, min(PREFETCH, SLOTS)):
            heads[j] = load_head(j)

        # HAM warmup fillers target the unused accumulator-bank columns so
        # the s-ring stays free for real S matmuls.  Interleaved with the
        # first chunks below to guarantee >2 full 4096-cycle windows of
        # continuous PE activity (the flip to full clock needs one fully
        # busy window; the fillers bridge early pipeline waits).
        warmA = a_ps.tile([128, 1024], F32, tag="accA", name="warmA")

        def filler(n):
            for _ in range(n):
                nc.tensor.matmul(
                    warmA[:, 272:512], warm[:, 0:128], warm[:, 0:240],
                    start=True, stop=True,
                )

        # global flattened software pipeline: S leads exp by 2 chunks,
        # PV lags exp by 2 chunks; no per-head drain bubbles.
        chunks = [(j, c) for j in range(SLOTS) for c in range(m_list[j])]
        NC_ = len(chunks)
        s_tiles: dict = {}
        pt_tiles: dict = {}
        accs: dict = {}

        def do_s(k):
            j, c = chunks[k]
            if c == 0 and j + PREFETCH < SLOTS and (j + PREFETCH) not in heads:
                heads[j + PREFETCH] = load_head(j + PREFETCH)
            qt, kk, _ = heads[j]
            s = s_ps.tile([128, L], F32, tag="s", name=f"s{j}_{c}")
            lo = 64 * (c % 2)
            blk = 128 * (c // 2)
            for h in range(2):
                nc.tensor.matmul(
                    s[:, 512 * h : 512 * h + 512],
                    kk[lo : lo + 64, blk : blk + 128],
                    qt[lo : lo + 64, 512 * h : 512 * h + 512],
                    start=True,
                    stop=True,
                )
            s_tiles[k] = s

        def do_exp(k, use_dve):
            j, c = chunks[k]
            pt = pt_p.tile([128, L], BF16, tag="pt", name=f"pt{j}_{c}")
            col = j * NCH + c
            s = s_tiles.pop(k)
            full = c < full_list[j]
            if use_dve:
                nc.vector.tensor_scalar(
                    pt.bitcast(I16)[:],
                    s[:],
                    SCH_A,
                    SCH_B if full else wv[:, col : col + 1],
                    Mult,
                    Add,
                )
            else:
                nc.scalar.activation(
                    pt[:],
                    s[:],
                    Exp,
                    bias=0.0 if full else mb[:, col : col + 1],
                    scale=0.125,
                )
            pt_tiles[k] = pt

        def do_pv(k):
            j, c = chunks[k]
            m = m_list[j]
            if c == 0:
                accs[j] = a_ps.tile([128, 1024], F32, tag="accA", name=f"acc{j}")
            acc = accs[j]
            pt = pt_tiles.pop(k)
            _, _, va = heads[j]
            for t in range(8):
                base = 512 * (t // 4) + 68 * (t % 4)
                # start=True resets has_written for the WHOLE bank, so only
                # the first matmul into each bank may set it; the other
                # slices' first writes overwrite (hw=false) anyway.
                nc.tensor.matmul(
                    acc[:, base : base + 65],
                    pt[:, 128 * t : 128 * t + 128],
                    va[:, 65 * c : 65 * c + 65],
                    start=(c == 0 and t in (0, 4)),
                    stop=(c == m - 1),
                )
            if c == m - 1:
                emit_epilogue(j)
                del heads[j]

        def emit_epilogue(j):
            acc = accs.pop(j)
            # [128, 2 banks, 4 tiles, 68] strided view of both acc banks
            acc4 = acc[:, 0:1024].rearrange("p (g x) -> p g x", g=2)[
                :, :, 0:272
            ].rearrange("p g (u e) -> p g u e", e=68)
            rec = rec_p.tile([128, 8], F32, tag="rec", name=f"rec{j}")
            rec3 = rec[:, 0:8].rearrange("p (g u) -> p g u", g=2)
            nc.vector.reciprocal(rec3, acc4[:, :, :, 64])
            osb = osb_p.tile([128, 512], F32, tag="osb", name=f"osb{j}")
            nc.vector.tensor_mul(
                osb[:, 0:512].rearrange("p (g u e) -> p g u e", g=2, e=64),
                acc4[:, :, :, 0:64],
                rec3.broadcast_to([128, 2, 4, 64]),
            )
            nc.gpsimd.dma_start(
                o_d[j].rearrange("(t p) d -> p t d", p=128),
                osb[:].rearrange("p (t d) -> p t d", d=64),
            )

        dve_flat = [dve[j][c] for j, c in chunks]
        filler(10)
        for k in range(min(2, NC_)):
            do_s(k)
            filler(4)
        for k in range(NC_):
            if 2 <= k < 8:
                filler(3)
            if k - 2 >= 0:
                do_pv(k - 2)
            if k + 2 < NC_:
                do_s(k + 2)
            do_exp(k, dve_flat[k])
        for k in range(max(0, NC_ - 2), NC_):
            do_pv(k)

    nc.compile()
    return nc


def _plan(valid_lens):
    """Sort heads by valid_len desc, deal round-robin across cores."""
    order = np.argsort(-valid_lens, kind="stable")
    assign = order.reshape(SLOTS, NCORES).T  # [core, slot]
    m_list = []
    full_list = []
    for j in range(SLOTS):
        vmax = int(valid_lens[assign[:, j]].max())
        vmin = int(valid_lens[assign[:, j]].min())
        m_list.append(min(NCH, max(1, math.ceil(vmax / CHUNK))))
        full_list.append(min(m_list[-1], vmin // CHUNK))
    return assign, m_list, full_list


def _prep_core(queries, keys, values, valid_lens, heads):
    qh = queries[heads]  # [SLOTS, L, D] f32
    kh = keys[heads]
    vh = values[heads]
    vl = valid_lens[heads]

    qt64 = np.transpose(qh, (0, 2, 1)).astype(np.float16)  # [j, d, q]
    qt = np.ascontiguousarray(np.concatenate([qt64, qt64], axis=1))  # [j, 128, q]

    kT = np.transpose(kh, (0, 2, 1)).astype(np.float16)  # [j, d, k]
    kT = kT.reshape(SLOTS, D, 4, 2, CHUNK)  # [j, d, blk, par, t]
    kk = np.ascontiguousarray(
        np.transpose(kT, (0, 3, 1, 2, 4)).reshape(SLOTS, 128, 512)
    )

    va0 = np.ones((SLOTS, NCH, CHUNK, 65), np.float32)
    va0[:, :, :, :64] = vh.reshape(SLOTS, NCH, CHUNK, D)
    va = np.ascontiguousarray(
        np.transpose(va0, (0, 2, 1, 3)).reshape(SLOTS, 128, NCH * 65)
    ).astype(ml_dtypes.bfloat16)

    kidx = np.arange(L).reshape(NCH, CHUNK)  # [c, p]
    valid = kidx[None] < vl[:, None, None]  # [j, c, p]
    mb = np.where(valid, 0.0, -1e6).astype(np.float32)
    mb = np.ascontiguousarray(np.transpose(mb, (2, 0, 1)).reshape(128, SLOTS * NCH))
    wv = np.where(valid, SCH_B, SCH_BMASK).astype(np.float32)
    wv = np.ascontiguousarray(np.transpose(wv, (2, 0, 1)).reshape(128, SLOTS * NCH))

    return {"qt": qt, "kk": kk, "va": va, "mb": mb, "wv": wv}


def _run(queries, keys, values, valid_lens, trace=False):
    queries = np.ascontiguousarray(np.asarray(queries, dtype=np.float32))
    keys = np.ascontiguousarray(np.asarray(keys, dtype=np.float32))
    values = np.ascontiguousarray(np.asarray(values, dtype=np.float32))
    valid_lens = np.asarray(valid_lens, dtype=np.int32)

    assign, m_list, full_list = _plan(valid_lens)

    key = (tuple(m_list), tuple(full_list))
    nc = _program_cache.get(key)
    if nc is None:
        nc = _build_program(m_list, full_list)
        _program_cache[key] = nc

    in_maps = [
        _prep_core(queries, keys, values, valid_lens, assign[i])
        for i in range(NCORES)
    ]

    res = run_bass_kernel_spmd(nc, in_maps, list(range(NCORES)), trace=trace)

    out = np.empty((BH, L, D), dtype=np.float32)
    for i in range(NCORES):
        out[assign[i]] = res.results[i]["o"]

    # valid_len == 0: reference softmaxes an all-masked row -> uniform weights.
    for h in np.nonzero(valid_lens == 0)[0]:
        out[h] = values[h].mean(axis=0, keepdims=True)

    return out, res


def kernel(queries, keys, values, valid_lens):
    out, _ = _run(queries, keys, values, valid_lens)
    return out


# revision 26
# speedup vs baseline: 1.0232x; 1.0232x over previous
"""Bass/Tile kernel for masked dot-product attention on 8 Trainium2 cores.

Problem: queries/keys/values [128, 1024, 64] fp32, valid_lens [128] int32.
  out[b] = softmax(mask(Q K^T / 8, valid_lens[b])) @ V

Design (v2):
  * Shard the 128 batch*heads across 8 cores, 16 head-slots per core.
    Heads sorted by valid_len desc, dealt round-robin -> one SPMD program.
  * Host pre-layout (numpy): Q^T/K^T transposed + fp16 cast on the host so
    the device needs no transposes at all.  Q^T is duplicated into both
    64-partition halves; K^T chunk pairs are packed into lower/upper halves
    so adjacent S matmuls row-tile (contraction=64 -> 2 concurrent).
  * V is packed per key-chunk as [128 keys, 64 V | 1.0] bf16 with the ones
    column baked in on the host; the PV matmul then yields [O | denom].
  * S^T = K_c Q^T per chunk on the PE -> PSUM [128 keys, 1024 q].
  * exp() is split between ScalarE (true Exp activation, masked via bias
    column) and VectorE (Schraudolph: one tensor_scalar computing the bf16
    BIT PATTERN of exp(s/8) as s*A + B[p] with int16 convert; masked keys
    get a bias that lands on a tiny positive bf16).  ~2x softmax throughput.
  * PV computes O directly in [q, d] layout: stationary = P^T q-tile
    [keys, 128], moving = [V_c | 1] -> accumulates [q, 64+1] in PSUM.
    No PE transposes, no epilogue copies; normalization is a reciprocal of
    the denom column + 8 per-partition-scalar multiplies.
  * Heads with valid_len == 0 are fixed up on the host (reference: uniform
    attention = mean of V).
"""

import math
from contextlib import ExitStack

import numpy as np
import ml_dtypes

import concourse.bass as bass  # noqa: F401
import concourse.mybir as mybir
import concourse.tile as tile
from concourse import bacc
from concourse.bass_utils import run_bass_kernel_spmd

BH, L, D = 128, 1024, 64
NCORES = 8
SLOTS = BH // NCORES  # 16
CHUNK = 128
NCH = L // CHUNK  # 8
F32 = mybir.dt.float32
F16 = mybir.dt.float16
BF16 = mybir.dt.bfloat16
I16 = mybir.dt.int16

# Schraudolph exp: bf16 bits of exp(s/8) ~= int16(s * SCH_A + SCH_B).
SCH_A = 0.125 * 1.4426950408889634 * 128.0  # 23.0831...
SCH_C = 6.0  # spread-centering correction (calibrated vs reference)
SCH_B = 127.0 * 128.0 - SCH_C
SCH_BMASK = 3100.0  # masked keys: packed in [~1540, ~4660] -> <2^-90, covers |s|<=134

DVE_FRAC = 0.36  # fraction of exp chunks routed to VectorE
PAIR_MODE = "none"  # 'all' | 'none' | 'alt' row-tiled S pairing

_program_cache: dict = {}


def _dve_pattern(m_list):
    """Deterministic ACT/DVE split per (slot, chunk), Bresenham on DVE_FRAC.

    The first two chunks of each head stay on ScalarE: their PV feeds the
    accumulator banks right when the previous head's epilogue occupies the
    VectorE queue, so a DVE exp there would serialize with it."""
    use = []
    err = 0.0
    for m in m_list:
        row = []
        for c in range(m):
            err += DVE_FRAC
            if err >= 1.0:
                err -= 1.0
                row.append(True)
            else:
                row.append(False)
        use.append(row)
    return use


def _build_program(m_list, full_list):
    dve = _dve_pattern(m_list)
    nc = bacc.Bacc("TRN2", target_bir_lowering=False, debug=False)
    qt_d = nc.dram_tensor("qt", [SLOTS, 128, L], F16, kind="ExternalInput").ap()
    kk_d = nc.dram_tensor("kk", [SLOTS, 128, 512], F16, kind="ExternalInput").ap()
    va_d = nc.dram_tensor("va", [SLOTS, 128, NCH * 65], BF16, kind="ExternalInput").ap()
    mb_d = nc.dram_tensor("mb", [128, SLOTS * NCH], F32, kind="ExternalInput").ap()
    wv_d = nc.dram_tensor("wv", [128, SLOTS * NCH], F32, kind="ExternalInput").ap()
    o_d = nc.dram_tensor("o", [SLOTS, L, D], F32, kind="ExternalOutput").ap()

    Exp = mybir.ActivationFunctionType.Exp
    Mult = mybir.AluOpType.mult
    Add = mybir.AluOpType.add

    with tile.TileContext(nc) as tc, ExitStack() as ctx:
        const = ctx.enter_context(tc.tile_pool(name="const", bufs=1))
        mb = const.tile([128, SLOTS * NCH], F32)
        wv = const.tile([128, SLOTS * NCH], F32)
        ones = const.tile([128, 1], F32)
        nc.gpsimd.memset(ones[:], 1.0)
        # Pre-load the exp table set so the first real activation is fast.
        actwarm = const.tile([128, 1], F32, tag="actwarm")
        nc.scalar.activation(actwarm[:], ones[:], Exp, bias=0.0, scale=1.0)
        warm = const.tile([128, 512], BF16, tag="warm")
        nc.vector.memset(warm[:], 0.5)

        qt_p = ctx.enter_context(tc.tile_pool(name="qt", bufs=4))
        kk_p = ctx.enter_context(tc.tile_pool(name="kk", bufs=4))
        va_p = ctx.enter_context(tc.tile_pool(name="va", bufs=4))
        pt_p = ctx.enter_context(tc.tile_pool(name="pt", bufs=8))
        osb_p = ctx.enter_context(tc.tile_pool(name="osb", bufs=2))
        rec_p = ctx.enter_context(tc.tile_pool(name="rec", bufs=2))

        # PSUM: 8 banks = s-ring 3 x [128,1024] (6) + two 1-bank accumulators.
        s_ps = ctx.enter_context(tc.tile_pool(name="s", bufs=3, space="PSUM"))
        a_ps = ctx.enter_context(tc.tile_pool(name="acc", bufs=1, space="PSUM"))

        def load_head(j):
            qt = qt_p.tile([128, L], F16, tag="qt", name=f"qt{j}")
            nc.sync.dma_start(qt[:], qt_d[j])
            kk = kk_p.tile([128, 512], F16, tag="kk", name=f"kk{j}")
            nc.sync.dma_start(kk[:], kk_d[j])
            va = va_p.tile([128, NCH * 65], BF16, tag="va", name=f"va{j}")
            nc.gpsimd.dma_start(va[:], va_d[j])
            return qt, kk, va

        PREFETCH = 3
        heads = {0: load_head(0)}
        nc.sync.dma_start(mb[:], mb_d[:])
        nc.sync.dma_start(wv[:], wv_d[:])
        for j in range(1, min(PREFETCH, SLOTS)):
            heads[j] = load_head(j)

        # HAM warmup fillers target the unused accumulator-bank columns so
        # the s-ring stays free for real S matmuls.  Interleaved with the
        # first chunks below to guarantee >2 full 4096-cycle windows of
        # continuous PE activity (the flip to full clock needs one fully
        # busy window; the fillers bridge early pipeline waits).
        warmA = a_ps.tile([128, 512], F32, tag="accA", name="warmA")

        def filler(n):
            for _ in range(n):
                nc.tensor.matmul(
                    warmA[:, 272:512], warm[:, 0:128], warm[:, 0:240],
                    start=True, stop=True,
                )

        # global flattened software pipeline: S leads exp by 2 chunks,
        # PV lags exp by 2 chunks; no per-head drain bubbles.
        chunks = [(j, c) for j in range(SLOTS) for c in range(m_list[j])]
        NC_ = len(chunks)
        s_tiles: dict = {}
        pt_tiles: dict = {}
        accs: dict = {}

        def do_s(k):
            j, c = chunks[k]
            if c == 0 and j + PREFETCH < SLOTS and (j + PREFETCH) not in heads:
                heads[j + PREFETCH] = load_head(j + PREFETCH)
            qt, kk, _ = heads[j]
            s = s_ps.tile([128, L], F32, tag="s", name=f"s{j}_{c}")
            lo = 64 * (c % 2)
            blk = 128 * (c // 2)
            for h in range(2):
                nc.tensor.matmul(
                    s[:, 512 * h : 512 * h + 512],
                    kk[lo : lo + 64, blk : blk + 128],
                    qt[lo : lo + 64, 512 * h : 512 * h + 512],
                    start=True,
                    stop=True,
                )
            s_tiles[k] = s

        def do_exp(k, use_dve):
            j, c = chunks[k]
            pt = pt_p.tile([128, L], BF16, tag="pt", name=f"pt{j}_{c}")
            col = j * NCH + c
            s = s_tiles.pop(k)
            full = c < full_list[j]
            if use_dve:
                nc.vector.tensor_scalar(
                    pt.bitcast(I16)[:],
                    s[:],
                    SCH_A,
                    SCH_B if full else wv[:, col : col + 1],
                    Mult,
                    Add,
                )
            else:
                nc.scalar.activation(
                    pt[:],
                    s[:],
                    Exp,
                    bias=0.0 if full else mb[:, col : col + 1],
                    scale=0.125,
                )
            pt_tiles[k] = pt

        def do_pv(k):
            j, c = chunks[k]
            m = m_list[j]
            if c == 0:
                accs[j] = (
                    a_ps.tile([128, 512], F32, tag="accA", name=f"accA{j}"),
                    a_ps.tile([128, 512], F32, tag="accB", name=f"accB{j}"),
                )
            accA, accB = accs[j]
            pt = pt_tiles.pop(k)
            _, _, va = heads[j]
            for t in range(8):
                av, u = (accA, t) if t < 4 else (accB, t - 4)
                # start=True resets has_written for the WHOLE bank, so only
                # the first matmul into each bank may set it; the other
                # slices' first writes overwrite (hw=false) anyway.
                nc.tensor.matmul(
                    av[:, 68 * u : 68 * u + 65],
                    pt[:, 128 * t : 128 * t + 128],
                    va[:, 65 * c : 65 * c + 65],
                    start=(c == 0 and t in (0, 4)),
                    stop=(c == m - 1),
                )
            if c == m - 1:
                emit_epilogue(j)
                del heads[j]

        def emit_epilogue(j):
            accA, accB = accs.pop(j)
            rec = rec_p.tile([128, 8], F32, tag="rec", name=f"rec{j}")
            nc.vector.reciprocal(
                rec[:, 0:4],
                accA[:, 0:272].rearrange("p (u e) -> p u e", e=68)[:, :, 64],
            )
            nc.vector.reciprocal(
                rec[:, 4:8],
                accB[:, 0:272].rearrange("p (u e) -> p u e", e=68)[:, :, 64],
            )
            osb = osb_p.tile([128, 512], F32, tag="osb", name=f"osb{j}")
            nc.vector.tensor_mul(
                osb[:, 0:256].rearrange("p (u e) -> p u e", e=64),
                accA[:, 0:272].rearrange("p (u e) -> p u e", e=68)[:, :, 0:64],
                rec[:, 0:4].broadcast_to([128, 4, 64]),
            )
            nc.vector.tensor_mul(
                osb[:, 256:512].rearrange("p (u e) -> p u e", e=64),
                accB[:, 0:272].rearrange("p (u e) -> p u e", e=68)[:, :, 0:64],
                rec[:, 4:8].broadcast_to([128, 4, 64]),
            )
            nc.gpsimd.dma_start(
                o_d[j].rearrange("(t p) d -> p t d", p=128),
                osb[:].rearrange("p (t d) -> p t d", d=64),
            )

        dve_flat = [dve[j][c] for j, c in chunks]
        filler(10)
        for k in range(min(2, NC_)):
            do_s(k)
            filler(4)
        for k in range(NC_):
            if 2 <= k < 8:
                filler(3)
            if k - 2 >= 0:
                do_pv(k - 2)
            if k + 2 < NC_:
                do_s(k + 2)
            do_exp(k, dve_flat[k])
        for k in range(max(0, NC_ - 2), NC_):
            do_pv(k)

    nc.compile()
    return nc


def _plan(valid_lens):
    """Sort heads by valid_len desc, deal round-robin across cores."""
    order = np.argsort(-valid_lens, kind="stable")
    assign = order.reshape(SLOTS, NCORES).T  # [core, slot]
    m_list = []
    full_list = []
    for j in range(SLOTS):
        vmax = int(valid_lens[assign[:, j]].max())
        vmin = int(valid_lens[assign[:, j]].min())
        m_list.append(min(NCH, max(1, math.ceil(vmax / CHUNK))))
        full_list.append(min(m_list[-1], vmin // CHUNK))
    return assign, m_list, full_list


def _prep_core(queries, keys, values, valid_lens, heads):
    qh = queries[heads]  # [SLOTS, L, D] f32
    kh = keys[heads]
    vh = values[heads]
    vl = valid_lens[heads]

    qt64 = np.transpose(qh, (0, 2, 1)).astype(np.float16)  # [j, d, q]
    qt = np.ascontiguousarray(np.concatenate([qt64, qt64], axis=1))  # [j, 128, q]

    kT = np.transpose(kh, (0, 2, 1)).astype(np.float16)  # [j, d, k]
    kT = kT.reshape(SLOTS, D, 4, 2, CHUNK)  # [j, d, blk, par, t]
    kk = np.ascontiguousarray(
        np.transpose(kT, (0, 3, 1, 2, 4)).reshape(SLOTS, 128, 512)
    )

    va0 = np.ones((SLOTS, NCH, CHUNK, 65), np.float32)
    va0[:, :, :, :64] = vh.reshape(SLOTS, NCH, CHUNK, D)
    va = np.ascontiguousarray(
        np.transpose(va0, (0, 2, 1, 3)).reshape(SLOTS, 128, NCH * 65)
    ).astype(ml_dtypes.bfloat16)

    kidx = np.arange(L).reshape(NCH, CHUNK)  # [c, p]
    valid = kidx[None] < vl[:, None, None]  # [j, c, p]
    mb = np.where(valid, 0.0, -1e6).astype(np.float32)
    mb = np.ascontiguousarray(np.transpose(mb, (2, 0, 1)).reshape(128, SLOTS * NCH))
    wv = np.where(valid, SCH_B, SCH_BMASK).astype(np.float32)
    wv = np.ascontiguousarray(np.transpose(wv, (2, 0, 1)).reshape(128, SLOTS * NCH))

    return {"qt": qt, "kk": kk, "va": va, "mb": mb, "wv": wv}


def _run(queries, keys, values, valid_lens, trace=False):
    queries = np.ascontiguousarray(np.asarray(queries, dtype=np.float32))
    keys = np.ascontiguousarray(np.asarray(keys, dtype=np.float32))
    values = np.ascontiguousarray(np.asarray(values, dtype=np.float32))
    valid_lens = np.asarray(valid_lens, dtype=np.int32)

    assign, m_list, full_list = _plan(valid_lens)

    key = (tuple(m_list), tuple(full_list))
    nc = _program_cache.get(key)
    if nc is None:
        nc = _build_program(m_list, full_list)
        _program_cache[key] = nc

    in_maps = [
        _prep_core(queries, keys, values, valid_lens, assign[i])
        for i in range(NCORES)
    ]

    res = run_bass_kernel_spmd(nc, in_maps, list(range(NCORES)), trace=trace)

    out = np.empty((BH, L, D), dtype=np.float32)
    for i in range(NCORES):
        out[assign[i]] = res.results[i]["o"]

    # valid_len == 0: reference softmaxes an all-masked row -> uniform weights.
    for h in np.nonzero(valid_lens == 0)[0]:
        out[h] = values[h].mean(axis=0, keepdims=True)

    return out, res


def kernel(queries, keys, values, valid_lens):
    out, _ = _run(queries, keys, values, valid_lens)
    return out


# revision 27
# speedup vs baseline: 1.0675x; 1.0433x over previous
"""Bass/Tile kernel for masked dot-product attention on 8 Trainium2 cores.

Problem: queries/keys/values [128, 1024, 64] fp32, valid_lens [128] int32.
  out[b] = softmax(mask(Q K^T / 8, valid_lens[b])) @ V

Design (v2):
  * Shard the 128 batch*heads across 8 cores, 16 head-slots per core.
    Heads sorted by valid_len desc, dealt round-robin -> one SPMD program.
  * Host pre-layout (numpy): Q^T/K^T transposed + fp16 cast on the host so
    the device needs no transposes at all.  Q^T is duplicated into both
    64-partition halves; K^T chunk pairs are packed into lower/upper halves
    so adjacent S matmuls row-tile (contraction=64 -> 2 concurrent).
  * V is packed per key-chunk as [128 keys, 64 V | 1.0] bf16 with the ones
    column baked in on the host; the PV matmul then yields [O | denom].
  * S^T = K_c Q^T per chunk on the PE -> PSUM [128 keys, 1024 q].
  * exp() is split between ScalarE (true Exp activation, masked via bias
    column) and VectorE (Schraudolph: one tensor_scalar computing the bf16
    BIT PATTERN of exp(s/8) as s*A + B[p] with int16 convert; masked keys
    get a bias that lands on a tiny positive bf16).  ~2x softmax throughput.
  * PV computes O directly in [q, d] layout: stationary = P^T q-tile
    [keys, 128], moving = [V_c | 1] -> accumulates [q, 64+1] in PSUM.
    No PE transposes, no epilogue copies; normalization is a reciprocal of
    the denom column + 8 per-partition-scalar multiplies.
  * Heads with valid_len == 0 are fixed up on the host (reference: uniform
    attention = mean of V).
"""

import math
from contextlib import ExitStack

import numpy as np
import ml_dtypes

import concourse.bass as bass  # noqa: F401
import concourse.mybir as mybir
import concourse.tile as tile
from concourse import bacc
from concourse.bass_utils import run_bass_kernel_spmd

BH, L, D = 128, 1024, 64
NCORES = 8
SLOTS = BH // NCORES  # 16
CHUNK = 128
NCH = L // CHUNK  # 8
F32 = mybir.dt.float32
F16 = mybir.dt.float16
BF16 = mybir.dt.bfloat16
I16 = mybir.dt.int16

# Schraudolph exp: bf16 bits of exp(s/8) ~= int16(s * SCH_A + SCH_B).
SCH_A = 0.125 * 1.4426950408889634 * 128.0  # 23.0831...
SCH_C = 6.0  # spread-centering correction (calibrated vs reference)
SCH_B = 127.0 * 128.0 - SCH_C
SCH_BMASK = 3100.0  # masked keys: packed in [~1540, ~4660] -> <2^-90, covers |s|<=134

DVE_FRAC = 0.36  # fraction of exp chunks routed to VectorE
PAIR_MODE = "none"  # 'all' | 'none' | 'alt' row-tiled S pairing

_program_cache: dict = {}


def _dve_pattern(m_list):
    """Deterministic ACT/DVE split per (slot, chunk), Bresenham on DVE_FRAC.

    The first two chunks of each head stay on ScalarE: their PV feeds the
    accumulator banks right when the previous head's epilogue occupies the
    VectorE queue, so a DVE exp there would serialize with it."""
    use = []
    err = 0.0
    for m in m_list:
        row = []
        for c in range(m):
            err += DVE_FRAC
            if err >= 1.0:
                err -= 1.0
                row.append(True)
            else:
                row.append(False)
        use.append(row)
    return use


def _build_program(m_list, full_list):
    dve = _dve_pattern(m_list)
    nc = bacc.Bacc("TRN2", target_bir_lowering=False, debug=False)
    qt_d = nc.dram_tensor("qt", [SLOTS, 128, L], F16, kind="ExternalInput").ap()
    kk_d = nc.dram_tensor("kk", [SLOTS, 128, 512], F16, kind="ExternalInput").ap()
    va_d = nc.dram_tensor("va", [SLOTS, 128, NCH * 65], BF16, kind="ExternalInput").ap()
    mb_d = nc.dram_tensor("mb", [128, SLOTS * NCH], F32, kind="ExternalInput").ap()
    wv_d = nc.dram_tensor("wv", [128, SLOTS * NCH], F32, kind="ExternalInput").ap()
    o_d = nc.dram_tensor("o", [SLOTS, L, D], F32, kind="ExternalOutput").ap()

    Exp = mybir.ActivationFunctionType.Exp
    Mult = mybir.AluOpType.mult
    Add = mybir.AluOpType.add

    with tile.TileContext(nc) as tc, ExitStack() as ctx:
        const = ctx.enter_context(tc.tile_pool(name="const", bufs=1))
        mb = const.tile([128, SLOTS * NCH], F32)
        wv = const.tile([128, SLOTS * NCH], F32)
        ones = const.tile([128, 1], F32)
        nc.gpsimd.memset(ones[:], 1.0)
        # Pre-load the exp table set so the first real activation is fast.
        actwarm = const.tile([128, 1], F32, tag="actwarm")
        nc.scalar.activation(actwarm[:], ones[:], Exp, bias=0.0, scale=1.0)
        warm = const.tile([128, 512], BF16, tag="warm")
        nc.vector.memset(warm[:], 0.5)

        qt_p = ctx.enter_context(tc.tile_pool(name="qt", bufs=4))
        kk_p = ctx.enter_context(tc.tile_pool(name="kk", bufs=4))
        va_p = ctx.enter_context(tc.tile_pool(name="va", bufs=4))
        pt_p = ctx.enter_context(tc.tile_pool(name="pt", bufs=8))
        osb_p = ctx.enter_context(tc.tile_pool(name="osb", bufs=2))
        rec_p = ctx.enter_context(tc.tile_pool(name="rec", bufs=2))

        # PSUM: 8 banks = s-ring 3 x [128,1024] (6) + two 1-bank accumulators.
        s_ps = ctx.enter_context(tc.tile_pool(name="s", bufs=3, space="PSUM"))
        a_ps = ctx.enter_context(tc.tile_pool(name="acc", bufs=1, space="PSUM"))

        def load_head(j):
            qt = qt_p.tile([128, L], F16, tag="qt", name=f"qt{j}")
            nc.sync.dma_start(qt[:], qt_d[j])
            kk = kk_p.tile([128, 512], F16, tag="kk", name=f"kk{j}")
            nc.sync.dma_start(kk[:], kk_d[j])
            va = va_p.tile([128, NCH * 65], BF16, tag="va", name=f"va{j}")
            nc.gpsimd.dma_start(va[:], va_d[j])
            return qt, kk, va

        PREFETCH = 3
        heads = {0: load_head(0)}
        nc.sync.dma_start(mb[:], mb_d[:])
        nc.sync.dma_start(wv[:], wv_d[:])
        for j in range(1, min(PREFETCH, SLOTS)):
            heads[j] = load_head(j)

        # HAM warmup fillers target the unused accumulator-bank columns so
        # the s-ring stays free for real S matmuls.  Interleaved with the
        # first chunks below to guarantee >2 full 4096-cycle windows of
        # continuous PE activity (the flip to full clock needs one fully
        # busy window; the fillers bridge early pipeline waits).
        warmA = a_ps.tile([128, 1024], F32, tag="accA", name="warmA")

        def filler(n):
            for _ in range(n):
                nc.tensor.matmul(
                    warmA[:, 272:512], warm[:, 0:128], warm[:, 0:240],
                    start=True, stop=True,
                )

        # global flattened software pipeline: S leads exp by 2 chunks,
        # PV lags exp by 2 chunks; no per-head drain bubbles.
        chunks = [(j, c) for j in range(SLOTS) for c in range(m_list[j])]
        NC_ = len(chunks)
        s_tiles: dict = {}
        pt_tiles: dict = {}
        accs: dict = {}

        def do_s(k):
            j, c = chunks[k]
            if c == 0 and j + PREFETCH < SLOTS and (j + PREFETCH) not in heads:
                heads[j + PREFETCH] = load_head(j + PREFETCH)
            qt, kk, _ = heads[j]
            s = s_ps.tile([128, L], F32, tag="s", name=f"s{j}_{c}")
            lo = 64 * (c % 2)
            blk = 128 * (c // 2)
            for h in range(2):
                nc.tensor.matmul(
                    s[:, 512 * h : 512 * h + 512],
                    kk[lo : lo + 64, blk : blk + 128],
                    qt[lo : lo + 64, 512 * h : 512 * h + 512],
                    start=True,
                    stop=True,
                )
            s_tiles[k] = s

        def do_exp(k, use_dve):
            j, c = chunks[k]
            pt = pt_p.tile([128, L], BF16, tag="pt", name=f"pt{j}_{c}")
            col = j * NCH + c
            s = s_tiles.pop(k)
            full = c < full_list[j]
            if use_dve:
                nc.vector.tensor_scalar(
                    pt.bitcast(I16)[:],
                    s[:],
                    SCH_A,
                    SCH_B if full else wv[:, col : col + 1],
                    Mult,
                    Add,
                )
            else:
                nc.scalar.activation(
                    pt[:],
                    s[:],
                    Exp,
                    bias=0.0 if full else mb[:, col : col + 1],
                    scale=0.125,
                )
            pt_tiles[k] = pt

        def do_pv(k):
            j, c = chunks[k]
            m = m_list[j]
            if c == 0:
                accs[j] = a_ps.tile([128, 1024], F32, tag="accA", name=f"acc{j}")
            acc = accs[j]
            pt = pt_tiles.pop(k)
            _, _, va = heads[j]
            for t in range(8):
                base = 512 * (t // 4) + 68 * (t % 4)
                # start=True resets has_written for the WHOLE bank, so only
                # the first matmul into each bank may set it; the other
                # slices' first writes overwrite (hw=false) anyway.
                nc.tensor.matmul(
                    acc[:, base : base + 65],
                    pt[:, 128 * t : 128 * t + 128],
                    va[:, 65 * c : 65 * c + 65],
                    start=(c == 0 and t in (0, 4)),
                    stop=(c == m - 1),
                )
            if c == m - 1:
                emit_epilogue(j)
                del heads[j]

        def emit_epilogue(j):
            acc = accs.pop(j)
            # [128, 2 banks, 4 tiles, 68] strided view over both acc banks
            acc4 = acc[:, 0:1024].rearrange("p (g x) -> p g x", g=2)[
                :, :, 0:272
            ].rearrange("p g (u e) -> p g u e", e=68)
            rec = rec_p.tile([128, 8], F32, tag="rec", name=f"rec{j}")
            rec3 = rec[:, 0:8].rearrange("p (g u) -> p g u", g=2)
            nc.vector.reciprocal(rec3, acc4[:, :, :, 64])
            osb = osb_p.tile([128, 512], F32, tag="osb", name=f"osb{j}")
            nc.vector.tensor_mul(
                osb[:, 0:512].rearrange("p (g u e) -> p g u e", g=2, e=64),
                acc4[:, :, :, 0:64],
                rec3.broadcast_to([128, 2, 4, 64]),
            )
            nc.gpsimd.dma_start(
                o_d[j].rearrange("(t p) d -> p t d", p=128),
                osb[:].rearrange("p (t d) -> p t d", d=64),
            )

        dve_flat = [dve[j][c] for j, c in chunks]
        filler(10)
        for k in range(min(2, NC_)):
            do_s(k)
            filler(4)
        for k in range(NC_):
            if 2 <= k < 8:
                filler(3)
            if k - 2 >= 0:
                do_pv(k - 2)
            if k + 2 < NC_:
                do_s(k + 2)
            do_exp(k, dve_flat[k])
        for k in range(max(0, NC_ - 2), NC_):
            do_pv(k)

    nc.compile()
    return nc


def _plan(valid_lens):
    """Sort heads by valid_len desc, deal round-robin across cores."""
    order = np.argsort(-valid_lens, kind="stable")
    assign = order.reshape(SLOTS, NCORES).T  # [core, slot]
    m_list = []
    full_list = []
    for j in range(SLOTS):
        vmax = int(valid_lens[assign[:, j]].max())
        vmin = int(valid_lens[assign[:, j]].min())
        m_list.append(min(NCH, max(1, math.ceil(vmax / CHUNK))))
        full_list.append(min(m_list[-1], vmin // CHUNK))
    return assign, m_list, full_list


def _prep_core(queries, keys, values, valid_lens, heads):
    qh = queries[heads]  # [SLOTS, L, D] f32
    kh = keys[heads]
    vh = values[heads]
    vl = valid_lens[heads]

    qt64 = np.transpose(qh, (0, 2, 1)).astype(np.float16)  # [j, d, q]
    qt = np.ascontiguousarray(np.concatenate([qt64, qt64], axis=1))  # [j, 128, q]

    kT = np.transpose(kh, (0, 2, 1)).astype(np.float16)  # [j, d, k]
    kT = kT.reshape(SLOTS, D, 4, 2, CHUNK)  # [j, d, blk, par, t]
    kk = np.ascontiguousarray(
        np.transpose(kT, (0, 3, 1, 2, 4)).reshape(SLOTS, 128, 512)
    )

    va0 = np.ones((SLOTS, NCH, CHUNK, 65), np.float32)
    va0[:, :, :, :64] = vh.reshape(SLOTS, NCH, CHUNK, D)
    va = np.ascontiguousarray(
        np.transpose(va0, (0, 2, 1, 3)).reshape(SLOTS, 128, NCH * 65)
    ).astype(ml_dtypes.bfloat16)

    kidx = np.arange(L).reshape(NCH, CHUNK)  # [c, p]
    valid = kidx[None] < vl[:, None, None]  # [j, c, p]
    mb = np.where(valid, 0.0, -1e6).astype(np.float32)
    mb = np.ascontiguousarray(np.transpose(mb, (2, 0, 1)).reshape(128, SLOTS * NCH))
    wv = np.where(valid, SCH_B, SCH_BMASK).astype(np.float32)
    wv = np.ascontiguousarray(np.transpose(wv, (2, 0, 1)).reshape(128, SLOTS * NCH))

    return {"qt": qt, "kk": kk, "va": va, "mb": mb, "wv": wv}


def _run(queries, keys, values, valid_lens, trace=False):
    queries = np.ascontiguousarray(np.asarray(queries, dtype=np.float32))
    keys = np.ascontiguousarray(np.asarray(keys, dtype=np.float32))
    values = np.ascontiguousarray(np.asarray(values, dtype=np.float32))
    valid_lens = np.asarray(valid_lens, dtype=np.int32)

    assign, m_list, full_list = _plan(valid_lens)

    key = (tuple(m_list), tuple(full_list))
    nc = _program_cache.get(key)
    if nc is None:
        nc = _build_program(m_list, full_list)
        _program_cache[key] = nc

    in_maps = [
        _prep_core(queries, keys, values, valid_lens, assign[i])
        for i in range(NCORES)
    ]

    res = run_bass_kernel_spmd(nc, in_maps, list(range(NCORES)), trace=trace)

    out = np.empty((BH, L, D), dtype=np.float32)
    for i in range(NCORES):
        out[assign[i]] = res.results[i]["o"]

    # valid_len == 0: reference softmaxes an all-masked row -> uniform weights.
    for h in np.nonzero(valid_lens == 0)[0]:
        out[h] = values[h].mean(axis=0, keepdims=True)

    return out, res


def kernel(queries, keys, values, valid_lens):
    out, _ = _run(queries, keys, values, valid_lens)
    return out
